# revision 24
# baseline (speedup 1.0000x reference)
"""Two-layer GraphSAGE (mean aggr) on 8 Trainium2 NeuronCores.

Strategy (1D graph partitioning by destination node):
  - Core k owns dst nodes [k*NPC, (k+1)*NPC) and all edges into them.
  - Aggregation per 128-node "bin": gather the source rows of the bin's
    edges with dma_gather (256B bf16 rows), build a per-bin 0/1 one-hot
    indicator on DVE (iota is_equal dstcol, bf16), and accumulate
    G.T @ onehot into PSUM on the TensorEngine -> unweighted msg_sum.T.
  - Mean normalization is folded out of the indicator: the whole bin row
    is scaled by 1/deg at the end.  To keep the self/bias terms correct
    under that scale, the lin_r input is pre-multiplied by deg on the
    host (xdegT) / on DVE (hdegT), and the bias is fed through a
    deg-row rank-1 matmul:
       h = relu( (1/deg) * ( msg_sum@Wl + (deg*x)@Wr + deg (x) b ) )
    The (1/deg) scale rides the Activation engine's per-partition scale
    operand of the relu/copy -- zero extra ops.
  - All tables bf16: gather rows are 256B (dma_gather minimum), matmuls
    run at 1 cycle/row (4x over fp32), AllGather ships half the bytes.
  - dma_gather indices are int16, so each gather table is split at row
    LOSPLIT=32768 (lo/hi); per-bin edge lists are sorted lo-then-hi and
    chunked into 128-edge chunks (padded with dc=-1 edges).
  - The indicator tile is laid out [128 edge, 128 col, nk chunk] (chunk
    LAST) so every DVE operand has a packed 2-byte last dim -> eligible
    for the DVE 2x perf mode.  The matmul rhs reads it with a strided
    free dim.
  - All per-core variation is input data (indices / columns / scales);
    the NEFF is one SPMD program. Chunk counts (K_lo/K_hi per layer) are
    derived from the actual graph at call time, then compiled.
"""

from dataclasses import dataclass

import numpy as np


@dataclass(frozen=True)
class Cfg:
    n_nodes: int = 50000
    d_in: int = 96
    d_hid: int = 128
    d_out: int = 128
    nc: int = 8
    lo_split: int = 32768
    call_ch: int = 32        # 128-edge chunks per dma_gather call
    n_queues: int = 4        # SWDGE queues (parallel Q7 descriptor gen)
    g_bufs: int = 4          # in-flight gather tiles
    ag_splits: int = 1       # chunked AllGather (overlap with layer-1 tail)
    reps: int = 1            # repeat whole computation in-NEFF (timing only)

    @property
    def npc(self):
        return self.n_nodes // self.nc

    @property
    def bins(self):
        return -(-self.npc // 128)

    @property
    def seg(self):
        return self.bins * 128

    @property
    def tbl(self):
        return self.seg * self.nc


DEFAULT_CFG = Cfg()


# ---------------------------------------------------------------- host side

def _bf16(a):
    import ml_dtypes
    return np.asarray(a, dtype=ml_dtypes.bfloat16)


def _wrap16(a):
    """Gather index layout: idx i -> [i % 16, i // 16], replicated 8x
    across the 128 partitions (one copy per Q7 core)."""
    return np.tile(a.reshape(-1, 16).T, (8, 1))


def _per_core_chunks(cfg, src_ids, cols, bins, k_lo, k_hi, split):
    """Arrange one core's edges (already split per bin) into the fixed
    chunk structure: per bin, k_lo lo-chunks then k_hi hi-chunks of 128
    edges. Returns (idx int16 [NCH*128], dc f32 [NCH*128]). idx is
    stream-major (lo block of bins*k_lo chunks, then the hi block); dc
    is bin-major (bin b's k_lo+k_hi chunks contiguous). Padding slots
    get idx 0 and dc -1 (no indicator match)."""
    n_bins = cfg.bins
    nk = k_lo + k_hi
    nch = n_bins * nk
    idx = np.zeros(nch * 128, dtype=np.int16)
    dc = np.full(nch * 128, -1.0, dtype=np.float32)
    n_lo_ch = n_bins * k_lo
    order = np.argsort(bins, kind="stable")
    src_ids, cols, bins = src_ids[order], cols[order], bins[order]
    bounds = np.searchsorted(bins, np.arange(n_bins + 1))
    for b in range(n_bins):
        s = src_ids[bounds[b]:bounds[b + 1]]
        c = cols[bounds[b]:bounds[b + 1]]
        lo = s < split
        for is_lo, kcap, idx_ch, dc_ch in (
                (True, k_lo, b * k_lo, b * nk),
                (False, k_hi, n_lo_ch + b * k_hi, b * nk + k_lo)):
            ss = s[lo] if is_lo else s[~lo] - split
            cc = c[lo] if is_lo else c[~lo]
            assert len(ss) <= kcap * 128, (len(ss), kcap)
            o = idx_ch * 128
            idx[o:o + len(ss)] = ss.astype(np.int16)
            o = dc_ch * 128
            dc[o:o + len(cc)] = cc.astype(np.float32)
    return idx, dc


def _pack_bins(n_bins, caps, vals, targets):
    """Greedy multi-dim balanced packing: assign each node (row of vals,
    [n_nodes, D]) to one of n_bins bins with slot capacity caps,
    minimizing the max over bins of each dimension's sum (normalized by
    targets). Nodes placed in decreasing total-weight order. Returns
    bin id per node."""
    n = vals.shape[0]
    order = np.argsort(-vals.sum(axis=1), kind="stable")
    sums = np.zeros((n_bins, vals.shape[1]))
    left = caps.astype(np.int64).copy()
    out = np.empty(n, dtype=np.int64)
    t = np.asarray(targets, dtype=np.float64)
    for i in order:
        score = ((sums + vals[i]) / t).max(axis=1)
        score[left <= 0] = np.inf
        b = int(np.argmin(score))
        out[i] = b
        sums[b] += vals[i]
        left[b] -= 1
    return out


def preprocess(cfg, x, edge_index, W1l, b1, W1r, W2l, b2, W2r):
    src = np.asarray(edge_index[0], dtype=np.int64)
    dst = np.asarray(edge_index[1], dtype=np.int64)
    x = np.asarray(x, dtype=np.float32)
    npc, n_bins = cfg.npc, cfg.bins
    lo_split2 = cfg.tbl // 2   # layer-2 h-table split (core-aligned: 4*seg)

    deg = np.bincount(dst, minlength=cfg.n_nodes).astype(np.float32)
    deg_c = np.maximum(deg, 1.0)
    inv_deg = (1.0 / deg_c).astype(np.float32)

    owner = dst // npc
    # per-node lo/hi degree for both layer splits (layer 2 splits the h
    # table at lo_split2 = 4*seg, i.e. source core < 4 -- independent of
    # the slot assignment below)
    lo1d = np.bincount(dst[src < cfg.lo_split], minlength=cfg.n_nodes)
    lo2d = np.bincount(dst[src < 4 * npc], minlength=cfg.n_nodes)
    degi = np.bincount(dst, minlength=cfg.n_nodes)

    # assign each core's dst nodes to (bin, col) slots, balancing the
    # four per-bin edge-count sums to minimize chunk padding
    perms = []          # per core: slot -> local node (len seg, -1 = empty)
    slot_of = np.empty(cfg.n_nodes, dtype=np.int64)   # node -> global slot
    for k in range(cfg.nc):
        lo1_k = lo1d[k * npc:(k + 1) * npc]
        lo2_k = lo2d[k * npc:(k + 1) * npc]
        deg_k = degi[k * npc:(k + 1) * npc]
        vals = np.stack([lo1_k, deg_k - lo1_k, lo2_k, deg_k - lo2_k],
                        axis=1).astype(np.float64)
        targets = np.maximum(vals.sum(axis=0) / n_bins, 1.0)
        caps = np.full(n_bins, 128)
        nbin = _pack_bins(n_bins, caps, vals, targets)
        order = np.argsort(nbin, kind="stable")
        perm = np.full(cfg.seg, -1, dtype=np.int64)
        cnt = np.bincount(nbin, minlength=n_bins)
        col = np.zeros(n_bins, dtype=np.int64)
        starts = np.arange(n_bins) * 128
        for ln in order:
            b = nbin[ln]
            s = starts[b] + col[b]
            col[b] += 1
            perm[s] = ln
            slot_of[k * npc + ln] = k * cfg.seg + s
        perms.append(perm)

    lslot = slot_of % cfg.seg       # slot within core
    ebin = lslot[dst] // 128
    ecol = lslot[dst] - ebin * 128
    pos = slot_of[src]              # row of src in h table

    per_core = []
    k1lo = k1hi = k2lo = k2hi = 1
    for k in range(cfg.nc):
        sel = owner == k
        s_k, b_k, c_k = src[sel], ebin[sel], ecol[sel]
        p_k = pos[sel]
        per_core.append((s_k, p_k, b_k, c_k))
        cnt = np.bincount(b_k, minlength=n_bins)
        lo1 = np.bincount(b_k[s_k < cfg.lo_split], minlength=n_bins)
        lo2 = np.bincount(b_k[p_k < lo_split2], minlength=n_bins)
        k1lo = max(k1lo, int(np.max(-(-lo1 // 128))))
        k1hi = max(k1hi, int(np.max(-(-(cnt - lo1) // 128))))
        k2lo = max(k2lo, int(np.max(-(-lo2 // 128))))
        k2hi = max(k2hi, int(np.max(-(-(cnt - lo2) // 128))))
    nkmax = max(k1lo + k1hi, k2lo + k2hi)

    xpad = np.zeros((cfg.n_nodes, 128), dtype=np.float32)
    xpad[:, :cfg.d_in] = x
    xpad = _bf16(xpad)
    # iota3[p, j, ci] = j  (chunk-last layout, materialized so every EQ
    # operand has a packed 2-byte last dim -> DVE 2x eligible)
    iota3 = _bf16(np.broadcast_to(np.arange(128, dtype=np.float32)[None, :, None],
                                  (128, 128, nkmax)).reshape(128, 128 * nkmax))
    ident = np.eye(128, dtype=np.float32)

    in_maps = []
    for k in range(cfg.nc):
        s_k, p_k, b_k, c_k = per_core[k]
        i1, d1 = _per_core_chunks(cfg, s_k, c_k, b_k, k1lo, k1hi,
                                  cfg.lo_split)
        i2, d2 = _per_core_chunks(cfg, p_k, c_k, b_k, k2lo, k2hi,
                                  lo_split2)
        # slot-ordered per-core tables (perm: slot -> local node)
        perm = perms[k]
        filled = perm >= 0
        nodes = np.clip(perm, 0, npc - 1) + k * npc
        deg_k = np.where(filled, deg_c[nodes], 1.0).astype(np.float32)
        inv_k = np.where(filled, inv_deg[nodes], 1.0).astype(np.float32)
        xdegT = np.where(filled[None, :],
                         (x[nodes] * deg_c[nodes, None]).T,
                         0.0).astype(np.float32)
        in_maps.append({
            "xpad": xpad,
            "xdegT": _bf16(xdegT),
            "idx1": np.ascontiguousarray(_wrap16(i1)),
            "dc1": _bf16(np.ascontiguousarray(d1.reshape(-1, 128).T)),
            "idx2": np.ascontiguousarray(_wrap16(i2)),
            "dc2": _bf16(np.ascontiguousarray(d2.reshape(-1, 128).T)),
            "W1l": _bf16(W1l),
            "W1r": _bf16(W1r),
            "W2l": _bf16(W2l),
            "W2r": _bf16(W2r),
            "b1row": _bf16(np.asarray(b1, np.float32).reshape(1, cfg.d_hid)),
            "b2row": _bf16(np.asarray(b2, np.float32).reshape(1, cfg.d_out)),
            "degrow": _bf16(deg_k.reshape(1, cfg.seg)),
            # column-per-bin tables for per-partition Act scales / DVE mult
            "scol": np.ascontiguousarray(inv_k.reshape(n_bins, 128).T),
            "dcol": np.ascontiguousarray(deg_k.reshape(n_bins, 128).T),
            "iota3": iota3,
            "ident": ident,
        })
    return in_maps, (k1lo, k1hi, k2lo, k2hi), perms


# -------------------------------------------------------------- device side

def build_program(cfg, k1lo, k1hi, k2lo, k2hi, debug_parts=frozenset()):
    """debug_parts: subset of {"no_l2", "no_gather", "no_mm", "no_eq",
    "no_ag"} to stub out components when bisecting hardware failures or
    attributing time (results are garbage for no_eq/no_ag)."""
    import concourse.bacc as bacc
    import concourse.tile as tile
    import concourse.mybir as mybir

    F32 = mybir.dt.float32
    BF16 = mybir.dt.bfloat16
    I16 = mybir.dt.int16
    EQ = mybir.AluOpType.is_equal
    MUL = mybir.AluOpType.mult
    RELU = mybir.ActivationFunctionType.Relu
    COPY = mybir.ActivationFunctionType.Copy

    n_bins, seg, tbl = cfg.bins, cfg.seg, cfg.tbl
    d_in, d_hid, d_out = cfg.d_in, cfg.d_hid, cfg.d_out
    nch1 = n_bins * (k1lo + k1hi)
    nch2 = n_bins * (k2lo + k2hi)
    nkmax = max(k1lo + k1hi, k2lo + k2hi)

    nc = bacc.Bacc("TRN2", target_bir_lowering=False, debug=False,
                   num_devices=cfg.nc, num_swdge_queues=cfg.n_queues)

    xpad = nc.dram_tensor("xpad", [cfg.n_nodes, 128], BF16, kind="ExternalInput")
    xdegT_d = nc.dram_tensor("xdegT", [d_in, seg], BF16, kind="ExternalInput")
    idx1_d = nc.dram_tensor("idx1", [128, nch1 * 8], I16, kind="ExternalInput")
    dc1_d = nc.dram_tensor("dc1", [128, nch1], BF16, kind="ExternalInput")
    idx2_d = nc.dram_tensor("idx2", [128, nch2 * 8], I16, kind="ExternalInput")
    dc2_d = nc.dram_tensor("dc2", [128, nch2], BF16, kind="ExternalInput")
    W1l_d = nc.dram_tensor("W1l", [d_in, d_hid], BF16, kind="ExternalInput")
    W1r_d = nc.dram_tensor("W1r", [d_in, d_hid], BF16, kind="ExternalInput")
    W2l_d = nc.dram_tensor("W2l", [d_hid, d_out], BF16, kind="ExternalInput")
    W2r_d = nc.dram_tensor("W2r", [d_hid, d_out], BF16, kind="ExternalInput")
    b1_d = nc.dram_tensor("b1row", [1, d_hid], BF16, kind="ExternalInput")
    b2_d = nc.dram_tensor("b2row", [1, d_out], BF16, kind="ExternalInput")
    degrow_d = nc.dram_tensor("degrow", [1, seg], BF16, kind="ExternalInput")
    scol_d = nc.dram_tensor("scol", [128, n_bins], F32, kind="ExternalInput")
    dcol_d = nc.dram_tensor("dcol", [128, n_bins], F32, kind="ExternalInput")
    iota3_d = nc.dram_tensor("iota3", [128, 128 * nkmax], BF16, kind="ExternalInput")
    ident_d = nc.dram_tensor("ident", [128, 128], F32, kind="ExternalInput")
    out_d = nc.dram_tensor("out", [seg, d_out], F32, kind="ExternalOutput")

    h_cc_in = nc.dram_tensor("h_cc_in", [seg, d_hid], BF16)
    h_full = nc.dram_tensor("h_full", [tbl, d_hid], BF16, addr_space="Shared")

    with tile.TileContext(nc) as tc:
        with tc.tile_pool(name="const", bufs=1) as cp, \
             tc.tile_pool(name="work", bufs=2) as wp, \
             tc.tile_pool(name="gtiles", bufs=cfg.g_bufs) as gp, \
             tc.tile_pool(name="ind", bufs=3) as ip, \
             tc.tile_pool(name="psA", bufs=3, space="PSUM") as psA, \
             tc.tile_pool(name="psB", bufs=3, space="PSUM") as psB, \
             tc.tile_pool(name="psC", bufs=2, space="PSUM") as psC:

            # ---- persistent constants
            xdegT = cp.tile([d_in, seg], BF16)
            hdegT = cp.tile([d_hid, seg], BF16)
            W1l = cp.tile([d_in, d_hid], BF16)
            W1r = cp.tile([d_in, d_hid], BF16)
            W2l = cp.tile([d_hid, d_out], BF16)
            W2r = cp.tile([d_hid, d_out], BF16)
            b1r = cp.tile([1, d_hid], BF16)
            b2r = cp.tile([1, d_out], BF16)
            degrow = cp.tile([1, seg], BF16)
            scol = cp.tile([128, n_bins], F32)
            dcol = cp.tile([128, n_bins], F32)
            iota3 = cp.tile([128, 128 * nkmax], BF16)
            ident = cp.tile([128, 128], F32)
            idx1 = cp.tile([128, nch1 * 8], I16)
            dc1 = cp.tile([128, nch1], BF16)
            idx2 = cp.tile([128, nch2 * 8], I16)
            dc2 = cp.tile([128, nch2], BF16)

            def load_constants():
                for t, d in ((xdegT, xdegT_d), (W1l, W1l_d), (W1r, W1r_d),
                             (W2l, W2l_d), (W2r, W2r_d), (b1r, b1_d),
                             (b2r, b2_d), (degrow, degrow_d), (scol, scol_d),
                             (dcol, dcol_d), (ident, ident_d), (iota3, iota3_d),
                             (dc1, dc1_d), (dc2, dc2_d)):
                    nc.sync.dma_start(t[:], d[:])
                # idx arrays on gpsimd so the gathers see them in program
                # order (dma_gather's idx read is NOT dep-tracked; loading
                # them on another engine breaks the gathers)
                nc.gpsimd.dma_start(idx1[:], idx1_d[:])
                nc.gpsimd.dma_start(idx2[:], idx2_d[:])

            iota3v = iota3[:].rearrange("p (a b) -> p a b", a=128)
            dummy_g = cp.tile([128, 1, 128], BF16)
            nc.vector.memset(dummy_g[:], 0.5)
            dummy_w = cp.tile([128, 128, nkmax], BF16)
            nc.vector.memset(dummy_w[:], 0.0)

            call_state = {"n": 0}

            def layer(klo, khi, idx_t, dc_t, lo_src, hi_src, df,
                      consume, tag, after_bin=None):
                n_lo_ch = n_bins * klo
                nk = klo + khi
                n_ch_stream = {0: n_lo_ch, 1: n_bins * khi}
                src = {0: lo_src, 1: hi_src}
                idx_base = {0: 0, 1: n_lo_ch}
                tiles = {}

                def chunk_tile(s, c):
                    """Gather tile slice holding stream-s chunk c, issuing
                    the covering dma_gather call on first touch."""
                    if "no_gather" in debug_parts:
                        return dummy_g[:, 0, :]
                    call = c // cfg.call_ch
                    if (s, call) not in tiles:
                        c0 = call * cfg.call_ch
                        n_c = min(cfg.call_ch, n_ch_stream[s] - c0)
                        G = gp.tile([128, cfg.call_ch, 128], BF16, tag="G")
                        q = call_state["n"] % cfg.n_queues
                        call_state["n"] += 1
                        ic0 = (idx_base[s] + c0) * 8
                        nc.gpsimd.dma_gather(
                            out_ap=G[:, 0:n_c, :], in_ap=src[s],
                            idxs_ap=idx_t[:, ic0:ic0 + n_c * 8],
                            num_idxs=n_c * 128, num_idxs_reg=n_c * 128,
                            elem_size=128, single_packet=False,
                            queue_num=q)
                        tiles[(s, call)] = G
                    return tiles[(s, call)][:, c - call * cfg.call_ch, :]

                for b in range(n_bins):
                    # 0/1 one-hot indicator for the bin's nk chunks,
                    # chunk-LAST layout: wind[p, j, ci] = (j == dc[p, ci])
                    if "no_eq" in debug_parts:
                        wind = dummy_w[:, :, 0:nk]
                    else:
                        wind = ip.tile([128, 128, nk], BF16, tag="wind")
                        nc.vector.tensor_tensor(
                            out=wind[:],
                            in0=iota3v[:, :, 0:nk],
                            in1=dc_t[:, b * nk:(b + 1) * nk]
                                .unsqueeze(1).broadcast_to((128, 128, nk)),
                            op=EQ)
                    pagg = psA.tile([df, 128], F32, tag="pagg")
                    no_mm = "no_mm" in debug_parts
                    for ci in range(nk):
                        if ci < klo:
                            s, c = 0, b * klo + ci
                        else:
                            s, c = 1, b * khi + (ci - klo)
                        g_sl = chunk_tile(s, c)[:, 0:df]
                        if no_mm and ci > 0:
                            continue
                        nc.tensor.matmul(pagg[:], lhsT=g_sl,
                                         rhs=wind[:, :, ci],
                                         start=(ci == 0),
                                         stop=(ci == nk - 1 or no_mm))
                    consume(b, pagg)
                    if after_bin is not None:
                        after_bin(b)

            # ---------------- layer 1
            def consume1(b, pagg):
                aggT = wp.tile([d_in, 128], BF16, tag="aggT")
                nc.scalar.copy(aggT[:], pagg[:])
                ph = psB.tile([128, d_hid], F32, tag="ph")
                nc.tensor.matmul(ph[:], lhsT=aggT[:], rhs=W1l[:],
                                 start=True, stop=False)
                nc.tensor.matmul(ph[:], lhsT=xdegT[:, b * 128:(b + 1) * 128],
                                 rhs=W1r[:], start=False, stop=False)
                nc.tensor.matmul(ph[:], lhsT=degrow[:, b * 128:(b + 1) * 128],
                                 rhs=b1r[:], start=False, stop=True)
                # h = relu(ph * (1/deg)) -- mean normalization folded into
                # the Activation per-partition scale
                h_t = wp.tile([128, d_hid], BF16, tag="h")
                nc.scalar.activation(h_t[:], ph[:], RELU,
                                     scale=scol[:, b:b + 1])
                nc.sync.dma_start(h_cc_in[b * 128:(b + 1) * 128, :], h_t[:])
                # hdeg = deg * h for layer 2's lin_r term (pre-scaled so the
                # final (1/deg) scale cancels)
                hdeg_t = wp.tile([128, d_hid], F32, tag="hdeg")
                nc.vector.tensor_scalar_mul(hdeg_t[:], h_t[:],
                                            dcol[:, b:b + 1])
                ptr = psC.tile([128, 128], F32, tag="ptr")
                nc.tensor.transpose(ptr[:], hdeg_t[:], ident[:])
                nc.scalar.copy(hdegT[:, b * 128:(b + 1) * 128], ptr[:])

            # ---------------- layer 2
            def consume2(b, pagg):
                agg2T = wp.tile([d_hid, 128], BF16, tag="agg2T")
                nc.scalar.copy(agg2T[:], pagg[:])
                po = psB.tile([128, d_out], F32, tag="ph")
                nc.tensor.matmul(po[:], lhsT=agg2T[:], rhs=W2l[:],
                                 start=True, stop=False)
                nc.tensor.matmul(po[:], lhsT=hdegT[:, b * 128:(b + 1) * 128],
                                 rhs=W2r[:], start=False, stop=False)
                nc.tensor.matmul(po[:], lhsT=degrow[:, b * 128:(b + 1) * 128],
                                 rhs=b2r[:], start=False, stop=True)
                o_t = wp.tile([128, d_out], F32, tag="o")
                nc.scalar.activation(o_t[:], po[:], COPY,
                                     scale=scol[:, b:b + 1])
                nc.sync.dma_start(out_d[b * 128:(b + 1) * 128, :], o_t[:])

            h_full_v = h_full[:].rearrange("(n s) d -> n s d", n=cfg.nc)

            def ag_slice(r0, r1):
                nc.gpsimd.collective_compute(
                    "AllGather", mybir.AluOpType.bypass,
                    replica_groups=[list(range(cfg.nc))],
                    ins=[h_cc_in[r0:r1, :]], outs=[h_full_v[:, r0:r1, :]])

            for _rep in range(cfg.reps):
                load_constants()
                # chunked AllGather: slice i ships as soon as its bins done
                S = cfg.ag_splits
                bounds = [(-(-n_bins * (i + 1) // S)) for i in range(S)]
                done = {b: i for i, b in enumerate(bounds)}

                def after1(b):
                    if "no_ag" in debug_parts or (b + 1) not in done:
                        return
                    i = done[b + 1]
                    r0 = 0 if i == 0 else bounds[i - 1] * 128
                    ag_slice(r0, bounds[i] * 128)

                layer(k1lo, k1hi, idx1, dc1, xpad[0:cfg.lo_split, :],
                      xpad[cfg.lo_split:cfg.n_nodes, :], d_in, consume1, "1",
                      after_bin=after1)

                if "no_ag" not in debug_parts:
                    bounce = wp.tile([1, d_hid], BF16, tag="bounce")
                    nc.gpsimd.dma_start(bounce[:], h_full[0:1, :])

                if "no_l2" in debug_parts:
                    for b in range(n_bins):
                        o_t = wp.tile([128, d_out], F32, tag="o")
                        nc.vector.tensor_copy(o_t[:],
                                              hdegT[:, b * 128:(b + 1) * 128])
                        nc.sync.dma_start(out_d[b * 128:(b + 1) * 128, :],
                                          o_t[:])
                else:
                    layer(k2lo, k2hi, idx2, dc2,
                          h_full[0:tbl // 2, :],
                          h_full[tbl // 2:tbl, :], d_hid, consume2, "2")

    nc.compile()
    return nc


_CACHE = {}


def run(cfg, inputs, _want_results=False, **spmd_kwargs):
    from concourse.bass_utils import run_bass_kernel_spmd

    in_maps, ks, perms = preprocess(cfg, **inputs)
    key = (cfg, ks)
    if key not in _CACHE:
        _CACHE[key] = build_program(cfg, *ks)
    nc = _CACHE[key]
    res = run_bass_kernel_spmd(nc, in_maps, core_ids=list(range(cfg.nc)),
                               **spmd_kwargs)
    npc = cfg.npc
    out = np.empty((cfg.n_nodes, cfg.d_out), dtype=np.float32)
    for k in range(cfg.nc):
        perm = perms[k]
        filled = perm >= 0
        out[k * npc + perm[filled]] = res.results[k]["out"][filled]
    if _want_results:
        return out, res
    return out


def kernel(x, edge_index, W1l, b1, W1r, W2l, b2, W2r):
    return run(DEFAULT_CFG, dict(x=x, edge_index=edge_index, W1l=W1l, b1=b1,
                                 W1r=W1r, W2l=W2l, b2=b2, W2r=W2r))


# revision 27
# speedup vs baseline: 1.2887x; 1.2887x over previous
"""Two-layer GraphSAGE (mean aggr) on 8 Trainium2 NeuronCores.

Strategy (1D graph partitioning by destination node):
  - Core k owns dst nodes [k*NPC, (k+1)*NPC) and all edges into them.
  - Aggregation per 128-node "bin": gather the source rows of the bin's
    edges with dma_gather (256B bf16 rows), build a per-bin 0/1 one-hot
    indicator on DVE (iota is_equal dstcol, bf16), and accumulate
    G.T @ onehot into PSUM on the TensorEngine -> unweighted msg_sum.T.
  - Mean normalization is folded out of the indicator: the whole bin row
    is scaled by 1/deg at the end.  To keep the self/bias terms correct
    under that scale, the lin_r input is pre-multiplied by deg on the
    host (xdegT) / on DVE (hdegT), and the bias is fed through a
    deg-row rank-1 matmul:
       h = relu( (1/deg) * ( msg_sum@Wl + (deg*x)@Wr + deg (x) b ) )
    The (1/deg) scale rides the Activation engine's per-partition scale
    operand of the relu/copy -- zero extra ops.
  - All tables bf16: gather rows are 256B (dma_gather minimum), matmuls
    run at 1 cycle/row (4x over fp32), AllGather ships half the bytes.
  - dma_gather indices are int16, so each gather table is split at row
    LOSPLIT=32768 (lo/hi); per-bin edge lists are sorted lo-then-hi and
    chunked into 128-edge chunks (padded with dc=-1 edges).
  - The indicator tile is laid out [128 edge, 128 col, nk chunk] (chunk
    LAST) so every DVE operand has a packed 2-byte last dim -> eligible
    for the DVE 2x perf mode.  The matmul rhs reads it with a strided
    free dim.
  - All per-core variation is input data (indices / columns / scales);
    the NEFF is one SPMD program. Chunk counts (K_lo/K_hi per layer) are
    derived from the actual graph at call time, then compiled.
"""

from dataclasses import dataclass

import numpy as np


@dataclass(frozen=True)
class Cfg:
    n_nodes: int = 50000
    d_in: int = 96
    d_hid: int = 128
    d_out: int = 128
    nc: int = 8
    lo_split: int = 32768
    call_ch: int = 16        # 128-edge chunks per dma_gather call
    n_queues: int = 4        # SWDGE queues (parallel Q7 descriptor gen)
    g_bufs: int = 16         # in-flight gather tiles (gather latency is the
                             # wall on HW; deep pipeline per lo/hi stream)
    ag_splits: int = 2       # chunked AllGather (overlap with layer-1 tail)
    reps: int = 1            # repeat whole computation in-NEFF (timing only)

    @property
    def npc(self):
        return self.n_nodes // self.nc

    @property
    def bins(self):
        return -(-self.npc // 128)

    @property
    def seg(self):
        return self.bins * 128

    @property
    def tbl(self):
        return self.seg * self.nc


DEFAULT_CFG = Cfg()


# ---------------------------------------------------------------- host side

def _bf16(a):
    import ml_dtypes
    return np.asarray(a, dtype=ml_dtypes.bfloat16)


def _wrap16(a):
    """Gather index layout: idx i -> [i % 16, i // 16], replicated 8x
    across the 128 partitions (one copy per Q7 core)."""
    return np.tile(a.reshape(-1, 16).T, (8, 1))


def _per_core_chunks(cfg, src_ids, cols, bins, k_lo, k_hi, split):
    """Arrange one core's edges (already split per bin) into the fixed
    chunk structure: per bin, k_lo lo-chunks then k_hi hi-chunks of 128
    edges. Returns (idx int16 [NCH*128], dc f32 [NCH*128]). idx is
    stream-major (lo block of bins*k_lo chunks, then the hi block); dc
    is bin-major (bin b's k_lo+k_hi chunks contiguous). Padding slots
    get idx 0 and dc -1 (no indicator match)."""
    n_bins = cfg.bins
    nk = k_lo + k_hi
    nch = n_bins * nk
    idx = np.zeros(nch * 128, dtype=np.int16)
    dc = np.full(nch * 128, -1.0, dtype=np.float32)
    n_lo_ch = n_bins * k_lo
    order = np.argsort(bins, kind="stable")
    src_ids, cols, bins = src_ids[order], cols[order], bins[order]
    bounds = np.searchsorted(bins, np.arange(n_bins + 1))
    for b in range(n_bins):
        s = src_ids[bounds[b]:bounds[b + 1]]
        c = cols[bounds[b]:bounds[b + 1]]
        lo = s < split
        for is_lo, kcap, idx_ch, dc_ch in (
                (True, k_lo, b * k_lo, b * nk),
                (False, k_hi, n_lo_ch + b * k_hi, b * nk + k_lo)):
            ss = s[lo] if is_lo else s[~lo] - split
            cc = c[lo] if is_lo else c[~lo]
            assert len(ss) <= kcap * 128, (len(ss), kcap)
            o = idx_ch * 128
            idx[o:o + len(ss)] = ss.astype(np.int16)
            o = dc_ch * 128
            dc[o:o + len(cc)] = cc.astype(np.float32)
    return idx, dc


def _pack_bins(n_bins, caps, vals, targets):
    """Greedy multi-dim balanced packing: assign each node (row of vals,
    [n_nodes, D]) to one of n_bins bins with slot capacity caps,
    minimizing the max over bins of each dimension's sum (normalized by
    targets). Nodes placed in decreasing total-weight order. Returns
    bin id per node."""
    n = vals.shape[0]
    order = np.argsort(-vals.sum(axis=1), kind="stable")
    sums = np.zeros((n_bins, vals.shape[1]))
    left = caps.astype(np.int64).copy()
    out = np.empty(n, dtype=np.int64)
    t = np.asarray(targets, dtype=np.float64)
    for i in order:
        score = ((sums + vals[i]) / t).max(axis=1)
        score[left <= 0] = np.inf
        b = int(np.argmin(score))
        out[i] = b
        sums[b] += vals[i]
        left[b] -= 1
    return out


def preprocess(cfg, x, edge_index, W1l, b1, W1r, W2l, b2, W2r):
    src = np.asarray(edge_index[0], dtype=np.int64)
    dst = np.asarray(edge_index[1], dtype=np.int64)
    x = np.asarray(x, dtype=np.float32)
    npc, n_bins = cfg.npc, cfg.bins
    lo_split2 = cfg.tbl // 2   # layer-2 h-table split (core-aligned: 4*seg)

    deg = np.bincount(dst, minlength=cfg.n_nodes).astype(np.float32)
    deg_c = np.maximum(deg, 1.0)
    inv_deg = (1.0 / deg_c).astype(np.float32)

    owner = dst // npc
    # per-node lo/hi degree for both layer splits. Layer 2 splits the h
    # table (block layout, see build_program) at tbl//2; for ag_splits=1
    # that's "source core < 4", known before packing. For ag_splits>1 the
    # split depends on the slot assignment itself, so balance on total
    # degree instead.
    lo1d = np.bincount(dst[src < cfg.lo_split], minlength=cfg.n_nodes)
    lo2d = np.bincount(dst[src < 4 * npc], minlength=cfg.n_nodes)
    degi = np.bincount(dst, minlength=cfg.n_nodes)

    # assign each core's dst nodes to (bin, col) slots, balancing the
    # per-bin edge-count sums to minimize chunk padding
    perms = []          # per core: slot -> local node (len seg, -1 = empty)
    lslot = np.empty(cfg.n_nodes, dtype=np.int64)   # node -> slot in core
    for k in range(cfg.nc):
        lo1_k = lo1d[k * npc:(k + 1) * npc]
        lo2_k = lo2d[k * npc:(k + 1) * npc]
        deg_k = degi[k * npc:(k + 1) * npc]
        if cfg.ag_splits == 1:
            vals = np.stack([lo1_k, deg_k - lo1_k, lo2_k, deg_k - lo2_k],
                            axis=1).astype(np.float64)
        else:
            vals = np.stack([lo1_k, deg_k - lo1_k, deg_k],
                            axis=1).astype(np.float64)
        targets = np.maximum(vals.sum(axis=0) / n_bins, 1.0)
        caps = np.full(n_bins, 128)
        nbin = _pack_bins(n_bins, caps, vals, targets)
        order = np.argsort(nbin, kind="stable")
        perm = np.full(cfg.seg, -1, dtype=np.int64)
        col = np.zeros(n_bins, dtype=np.int64)
        starts = np.arange(n_bins) * 128
        for ln in order:
            b = nbin[ln]
            s = starts[b] + col[b]
            col[b] += 1
            perm[s] = ln
            lslot[k * npc + ln] = s
        perms.append(perm)

    # h_full block layout: row of (core k, slot s) with blk = seg/S blocks
    blk = cfg.seg // cfg.ag_splits
    slot_of = ((lslot // blk) * cfg.nc * blk
               + (np.arange(cfg.n_nodes) // npc) * blk + lslot % blk)
    ebin = lslot[dst] // 128
    ecol = lslot[dst] - ebin * 128
    pos = slot_of[src]              # row of src in h table

    per_core = []
    k1lo = k1hi = k2lo = k2hi = 1
    for k in range(cfg.nc):
        sel = owner == k
        s_k, b_k, c_k = src[sel], ebin[sel], ecol[sel]
        p_k = pos[sel]
        per_core.append((s_k, p_k, b_k, c_k))
        cnt = np.bincount(b_k, minlength=n_bins)
        lo1 = np.bincount(b_k[s_k < cfg.lo_split], minlength=n_bins)
        lo2 = np.bincount(b_k[p_k < lo_split2], minlength=n_bins)
        k1lo = max(k1lo, int(np.max(-(-lo1 // 128))))
        k1hi = max(k1hi, int(np.max(-(-(cnt - lo1) // 128))))
        k2lo = max(k2lo, int(np.max(-(-lo2 // 128))))
        k2hi = max(k2hi, int(np.max(-(-(cnt - lo2) // 128))))
    nkmax = max(k1lo + k1hi, k2lo + k2hi)

    xpad = np.zeros((cfg.n_nodes, 128), dtype=np.float32)
    xpad[:, :cfg.d_in] = x
    xpad = _bf16(xpad)
    # iota3[p, j, ci] = j  (chunk-last layout, materialized so every EQ
    # operand has a packed 2-byte last dim -> DVE 2x eligible)
    iota3 = _bf16(np.broadcast_to(np.arange(128, dtype=np.float32)[None, :, None],
                                  (128, 128, nkmax)).reshape(128, 128 * nkmax))
    ident = np.eye(128, dtype=np.float32)

    in_maps = []
    for k in range(cfg.nc):
        s_k, p_k, b_k, c_k = per_core[k]
        i1, d1 = _per_core_chunks(cfg, s_k, c_k, b_k, k1lo, k1hi,
                                  cfg.lo_split)
        i2, d2 = _per_core_chunks(cfg, p_k, c_k, b_k, k2lo, k2hi,
                                  lo_split2)
        # slot-ordered per-core tables (perm: slot -> local node)
        perm = perms[k]
        filled = perm >= 0
        nodes = np.clip(perm, 0, npc - 1) + k * npc
        deg_k = np.where(filled, deg_c[nodes], 1.0).astype(np.float32)
        inv_k = np.where(filled, inv_deg[nodes], 1.0).astype(np.float32)
        xdegT = np.where(filled[None, :],
                         (x[nodes] * deg_c[nodes, None]).T,
                         0.0).astype(np.float32)
        in_maps.append({
            "xpad": xpad,
            "xdegT": _bf16(xdegT),
            "idx1": np.ascontiguousarray(_wrap16(i1)),
            "dc1": _bf16(np.ascontiguousarray(d1.reshape(-1, 128).T)),
            "idx2": np.ascontiguousarray(_wrap16(i2)),
            "dc2": _bf16(np.ascontiguousarray(d2.reshape(-1, 128).T)),
            "W1l": _bf16(W1l),
            "W1r": _bf16(W1r),
            "W2l": _bf16(W2l),
            "W2r": _bf16(W2r),
            "b1row": _bf16(np.asarray(b1, np.float32).reshape(1, cfg.d_hid)),
            "b2row": _bf16(np.asarray(b2, np.float32).reshape(1, cfg.d_out)),
            "degrow": _bf16(deg_k.reshape(1, cfg.seg)),
            # column-per-bin tables for per-partition Act scales / DVE mult
            "scol": np.ascontiguousarray(inv_k.reshape(n_bins, 128).T),
            "dcol": np.ascontiguousarray(deg_k.reshape(n_bins, 128).T),
            "iota3": iota3,
            "ident": ident,
        })
    return in_maps, (k1lo, k1hi, k2lo, k2hi), perms


# -------------------------------------------------------------- device side

def build_program(cfg, k1lo, k1hi, k2lo, k2hi, debug_parts=frozenset()):
    """debug_parts: subset of {"no_l2", "no_gather", "no_mm", "no_eq",
    "no_ag"} to stub out components when bisecting hardware failures or
    attributing time (results are garbage for no_eq/no_ag)."""
    import concourse.bacc as bacc
    import concourse.tile as tile
    import concourse.mybir as mybir

    F32 = mybir.dt.float32
    BF16 = mybir.dt.bfloat16
    I16 = mybir.dt.int16
    EQ = mybir.AluOpType.is_equal
    MUL = mybir.AluOpType.mult
    RELU = mybir.ActivationFunctionType.Relu
    COPY = mybir.ActivationFunctionType.Copy

    n_bins, seg, tbl = cfg.bins, cfg.seg, cfg.tbl
    d_in, d_hid, d_out = cfg.d_in, cfg.d_hid, cfg.d_out
    nch1 = n_bins * (k1lo + k1hi)
    nch2 = n_bins * (k2lo + k2hi)
    nkmax = max(k1lo + k1hi, k2lo + k2hi)

    nc = bacc.Bacc("TRN2", target_bir_lowering=False, debug=False,
                   num_devices=cfg.nc, num_swdge_queues=cfg.n_queues)

    xpad = nc.dram_tensor("xpad", [cfg.n_nodes, 128], BF16, kind="ExternalInput")
    xdegT_d = nc.dram_tensor("xdegT", [d_in, seg], BF16, kind="ExternalInput")
    idx1_d = nc.dram_tensor("idx1", [128, nch1 * 8], I16, kind="ExternalInput")
    dc1_d = nc.dram_tensor("dc1", [128, nch1], BF16, kind="ExternalInput")
    idx2_d = nc.dram_tensor("idx2", [128, nch2 * 8], I16, kind="ExternalInput")
    dc2_d = nc.dram_tensor("dc2", [128, nch2], BF16, kind="ExternalInput")
    W1l_d = nc.dram_tensor("W1l", [d_in, d_hid], BF16, kind="ExternalInput")
    W1r_d = nc.dram_tensor("W1r", [d_in, d_hid], BF16, kind="ExternalInput")
    W2l_d = nc.dram_tensor("W2l", [d_hid, d_out], BF16, kind="ExternalInput")
    W2r_d = nc.dram_tensor("W2r", [d_hid, d_out], BF16, kind="ExternalInput")
    b1_d = nc.dram_tensor("b1row", [1, d_hid], BF16, kind="ExternalInput")
    b2_d = nc.dram_tensor("b2row", [1, d_out], BF16, kind="ExternalInput")
    degrow_d = nc.dram_tensor("degrow", [1, seg], BF16, kind="ExternalInput")
    scol_d = nc.dram_tensor("scol", [128, n_bins], F32, kind="ExternalInput")
    dcol_d = nc.dram_tensor("dcol", [128, n_bins], F32, kind="ExternalInput")
    iota3_d = nc.dram_tensor("iota3", [128, 128 * nkmax], BF16, kind="ExternalInput")
    ident_d = nc.dram_tensor("ident", [128, 128], F32, kind="ExternalInput")
    out_d = nc.dram_tensor("out", [seg, d_out], F32, kind="ExternalOutput")

    h_cc_in = nc.dram_tensor("h_cc_in", [seg, d_hid], BF16)
    h_full = nc.dram_tensor("h_full", [tbl, d_hid], BF16, addr_space="Shared")

    with tile.TileContext(nc) as tc:
        with tc.tile_pool(name="const", bufs=1) as cp, \
             tc.tile_pool(name="work", bufs=2) as wp, \
             tc.tile_pool(name="gtiles", bufs=cfg.g_bufs) as gp, \
             tc.tile_pool(name="ind", bufs=3) as ip, \
             tc.tile_pool(name="psA", bufs=3, space="PSUM") as psA, \
             tc.tile_pool(name="psB", bufs=3, space="PSUM") as psB, \
             tc.tile_pool(name="psC", bufs=2, space="PSUM") as psC:

            # ---- persistent constants
            xdegT = cp.tile([d_in, seg], BF16)
            hdegT = cp.tile([d_hid, seg], BF16)
            W1l = cp.tile([d_in, d_hid], BF16)
            W1r = cp.tile([d_in, d_hid], BF16)
            W2l = cp.tile([d_hid, d_out], BF16)
            W2r = cp.tile([d_hid, d_out], BF16)
            b1r = cp.tile([1, d_hid], BF16)
            b2r = cp.tile([1, d_out], BF16)
            degrow = cp.tile([1, seg], BF16)
            scol = cp.tile([128, n_bins], F32)
            dcol = cp.tile([128, n_bins], F32)
            iota3 = cp.tile([128, 128 * nkmax], BF16)
            ident = cp.tile([128, 128], F32)
            idx1 = cp.tile([128, nch1 * 8], I16)
            dc1 = cp.tile([128, nch1], BF16)
            idx2 = cp.tile([128, nch2 * 8], I16)
            dc2 = cp.tile([128, nch2], BF16)

            def load_constants():
                for t, d in ((xdegT, xdegT_d), (W1l, W1l_d), (W1r, W1r_d),
                             (W2l, W2l_d), (W2r, W2r_d), (b1r, b1_d),
                             (b2r, b2_d), (degrow, degrow_d), (scol, scol_d),
                             (dcol, dcol_d), (ident, ident_d), (iota3, iota3_d),
                             (dc1, dc1_d), (dc2, dc2_d)):
                    nc.sync.dma_start(t[:], d[:])
                # idx arrays on gpsimd so the gathers see them in program
                # order (dma_gather's idx read is NOT dep-tracked; loading
                # them on another engine breaks the gathers)
                nc.gpsimd.dma_start(idx1[:], idx1_d[:])
                nc.gpsimd.dma_start(idx2[:], idx2_d[:])

            iota3v = iota3[:].rearrange("p (a b) -> p a b", a=128)
            dummy_g = cp.tile([128, 1, 128], BF16)
            nc.vector.memset(dummy_g[:], 0.5)
            dummy_w = cp.tile([128, 128, nkmax], BF16)
            nc.vector.memset(dummy_w[:], 0.0)

            call_state = {"n": 0}

            def layer(klo, khi, idx_t, dc_t, lo_src, hi_src, df,
                      consume, tag, after_bin=None):
                n_lo_ch = n_bins * klo
                nk = klo + khi
                n_ch_stream = {0: n_lo_ch, 1: n_bins * khi}
                src = {0: lo_src, 1: hi_src}
                idx_base = {0: 0, 1: n_lo_ch}
                tiles = {}

                def chunk_tile(s, c):
                    """Gather tile slice holding stream-s chunk c, issuing
                    the covering dma_gather call on first touch."""
                    if "no_gather" in debug_parts:
                        return dummy_g[:, 0, :]
                    call = c // cfg.call_ch
                    if (s, call) not in tiles:
                        c0 = call * cfg.call_ch
                        n_c = min(cfg.call_ch, n_ch_stream[s] - c0)
                        G = gp.tile([128, cfg.call_ch, 128], BF16, tag="G")
                        q = call_state["n"] % cfg.n_queues
                        call_state["n"] += 1
                        ic0 = (idx_base[s] + c0) * 8
                        nc.gpsimd.dma_gather(
                            out_ap=G[:, 0:n_c, :], in_ap=src[s],
                            idxs_ap=idx_t[:, ic0:ic0 + n_c * 8],
                            num_idxs=n_c * 128, num_idxs_reg=n_c * 128,
                            elem_size=128, single_packet=False,
                            queue_num=q)
                        tiles[(s, call)] = G
                    return tiles[(s, call)][:, c - call * cfg.call_ch, :]

                for b in range(n_bins):
                    # 0/1 one-hot indicator for the bin's nk chunks,
                    # chunk-LAST layout: wind[p, j, ci] = (j == dc[p, ci])
                    if "no_eq" in debug_parts:
                        wind = dummy_w[:, :, 0:nk]
                    else:
                        wind = ip.tile([128, 128, nk], BF16, tag="wind")
                        nc.vector.tensor_tensor(
                            out=wind[:],
                            in0=iota3v[:, :, 0:nk],
                            in1=dc_t[:, b * nk:(b + 1) * nk]
                                .unsqueeze(1).broadcast_to((128, 128, nk)),
                            op=EQ)
                    pagg = psA.tile([df, 128], F32, tag="pagg")
                    no_mm = "no_mm" in debug_parts
                    for ci in range(nk):
                        if ci < klo:
                            s, c = 0, b * klo + ci
                        else:
                            s, c = 1, b * khi + (ci - klo)
                        g_sl = chunk_tile(s, c)[:, 0:df]
                        if no_mm and ci > 0:
                            continue
                        nc.tensor.matmul(pagg[:], lhsT=g_sl,
                                         rhs=wind[:, :, ci],
                                         start=(ci == 0),
                                         stop=(ci == nk - 1 or no_mm))
                    consume(b, pagg)
                    if after_bin is not None:
                        after_bin(b)

            # ---------------- layer 1
            def consume1(b, pagg):
                aggT = wp.tile([d_in, 128], BF16, tag="aggT")
                nc.scalar.copy(aggT[:], pagg[:])
                ph = psB.tile([128, d_hid], F32, tag="ph")
                nc.tensor.matmul(ph[:], lhsT=aggT[:], rhs=W1l[:],
                                 start=True, stop=False)
                nc.tensor.matmul(ph[:], lhsT=xdegT[:, b * 128:(b + 1) * 128],
                                 rhs=W1r[:], start=False, stop=False)
                nc.tensor.matmul(ph[:], lhsT=degrow[:, b * 128:(b + 1) * 128],
                                 rhs=b1r[:], start=False, stop=True)
                # h = relu(ph * (1/deg)) -- mean normalization folded into
                # the Activation per-partition scale
                h_t = wp.tile([128, d_hid], BF16, tag="h")
                nc.scalar.activation(h_t[:], ph[:], RELU,
                                     scale=scol[:, b:b + 1])
                nc.sync.dma_start(h_cc_in[b * 128:(b + 1) * 128, :], h_t[:])
                # hdeg = deg * h for layer 2's lin_r term (pre-scaled so the
                # final (1/deg) scale cancels)
                hdeg_t = wp.tile([128, d_hid], F32, tag="hdeg")
                nc.vector.tensor_scalar_mul(hdeg_t[:], h_t[:],
                                            dcol[:, b:b + 1])
                ptr = psC.tile([128, 128], F32, tag="ptr")
                nc.tensor.transpose(ptr[:], hdeg_t[:], ident[:])
                nc.scalar.copy(hdegT[:, b * 128:(b + 1) * 128], ptr[:])

            # ---------------- layer 2
            def consume2(b, pagg):
                agg2T = wp.tile([d_hid, 128], BF16, tag="agg2T")
                nc.scalar.copy(agg2T[:], pagg[:])
                po = psB.tile([128, d_out], F32, tag="ph")
                nc.tensor.matmul(po[:], lhsT=agg2T[:], rhs=W2l[:],
                                 start=True, stop=False)
                nc.tensor.matmul(po[:], lhsT=hdegT[:, b * 128:(b + 1) * 128],
                                 rhs=W2r[:], start=False, stop=False)
                nc.tensor.matmul(po[:], lhsT=degrow[:, b * 128:(b + 1) * 128],
                                 rhs=b2r[:], start=False, stop=True)
                o_t = wp.tile([128, d_out], F32, tag="o")
                nc.scalar.activation(o_t[:], po[:], COPY,
                                     scale=scol[:, b:b + 1])
                nc.sync.dma_start(out_d[b * 128:(b + 1) * 128, :], o_t[:])

            # h_full block layout: slice i of the chunked AllGather outputs
            # the CONTIGUOUS range [i*nc*blk, (i+1)*nc*blk) (concat over
            # cores of h_cc_in[i*blk:(i+1)*blk]) -- no strided collective
            # APs. Host-side pos mapping uses the same layout.
            S = cfg.ag_splits
            blk = seg // S

            def ag_slice(i):
                nc.gpsimd.collective_compute(
                    "AllGather", mybir.AluOpType.bypass,
                    replica_groups=[list(range(cfg.nc))],
                    ins=[h_cc_in[i * blk:(i + 1) * blk, :]],
                    outs=[h_full[i * cfg.nc * blk:(i + 1) * cfg.nc * blk, :]])

            for _rep in range(cfg.reps):
                load_constants()
                # chunked AllGather: slice i ships as soon as its bins done
                done = {}
                for i in range(S):
                    done[-(-blk * (i + 1) // 128) - 1] = i

                def after1(b):
                    if "no_ag" in debug_parts or b not in done:
                        return
                    ag_slice(done[b])

                layer(k1lo, k1hi, idx1, dc1, xpad[0:cfg.lo_split, :],
                      xpad[cfg.lo_split:cfg.n_nodes, :], d_in, consume1, "1",
                      after_bin=after1)

                if "no_ag" not in debug_parts:
                    bounce = wp.tile([1, d_hid], BF16, tag="bounce")
                    nc.gpsimd.dma_start(bounce[:], h_full[0:1, :])

                if "no_l2" in debug_parts:
                    for b in range(n_bins):
                        o_t = wp.tile([128, d_out], F32, tag="o")
                        nc.vector.tensor_copy(o_t[:],
                                              hdegT[:, b * 128:(b + 1) * 128])
                        nc.sync.dma_start(out_d[b * 128:(b + 1) * 128, :],
                                          o_t[:])
                else:
                    layer(k2lo, k2hi, idx2, dc2,
                          h_full[0:tbl // 2, :],
                          h_full[tbl // 2:tbl, :], d_hid, consume2, "2")

    nc.compile()
    return nc


_CACHE = {}


def run(cfg, inputs, _want_results=False, **spmd_kwargs):
    from concourse.bass_utils import run_bass_kernel_spmd

    in_maps, ks, perms = preprocess(cfg, **inputs)
    key = (cfg, ks)
    if key not in _CACHE:
        _CACHE[key] = build_program(cfg, *ks)
    nc = _CACHE[key]
    res = run_bass_kernel_spmd(nc, in_maps, core_ids=list(range(cfg.nc)),
                               **spmd_kwargs)
    npc = cfg.npc
    out = np.empty((cfg.n_nodes, cfg.d_out), dtype=np.float32)
    for k in range(cfg.nc):
        perm = perms[k]
        filled = perm >= 0
        out[k * npc + perm[filled]] = res.results[k]["out"][filled]
    if _want_results:
        return out, res
    return out


def kernel(x, edge_index, W1l, b1, W1r, W2l, b2, W2r):
    return run(DEFAULT_CFG, dict(x=x, edge_index=edge_index, W1l=W1l, b1=b1,
                                 W1r=W1r, W2l=W2l, b2=b2, W2r=W2r))
